# revision 22
# baseline (speedup 1.0000x reference)
"""Trainium2 Bass kernel for nn_AttentionTypeEnsembleSheafLearner (v2).

Reference computation (per edge e with endpoints (r, c) and type t):
    h   = concat(x[r], x[c])                # [2C] = [256]
    mu, var = mean/var over the 256 features (non-affine LN stats)
    xh  = (h - mu) * rsqrt(var + eps)
    h1  = relu((xh * gamma[t] + beta[t]) @ W1[t] + b1[t])   # [64]
    o   = h1 @ W2[t] + b2[t]                                # [16]
    out = I4 - softmax(o.reshape(4,4), axis=-1)

v2 strategy (8 NeuronCores, data-parallel over edges):
  * Edges are dealt round-robin across cores, then grouped per core by
    (type, endpoint-range class).  The class splits edges by whether the
    r/c node ids are < 32768 (dma_gather's int16 index limit), so each
    segment can gather its endpoint rows with batched dma_gather calls
    (transpose=True) from the matching half of a bf16 copy of x.  The
    gather lands feature-major ([128 feats, edges]) — no PE transposes.
  * LayerNorm is folded into the matmuls: with std_e = sqrt(var+eps) and
    total[h,e] = (W1e^T h_cat)[h,e] - mu_e*u[h] + std_e*b1e[h]
    (u = column sums of W1e), we have  z = inv_e * total  and since
    inv_e > 0:  relu(z) = inv_e * relu(total).  The -mu/std terms ride a
    K=2 matmul chunk; inv_e is applied after mm2 (where edges sit on
    partitions) as a per-partition scalar multiply.  b2 rides an
    augmented K=65 row of mm2 scaled by std_e so it survives the final
    inv multiply exactly.
  * All matmul operands bf16 (tolerance 2e-2; measured ~1e-3), PSUM f32.
  * Softmax + (I - attn) run as a few mega-batched DVE/Act ops.
"""

import math
import os
import sys

import numpy as np
import ml_dtypes

for _p in ("/opt/trn_rl_repo",):
    if _p not in sys.path:
        sys.path.insert(0, _p)

bf16 = ml_dtypes.bfloat16

# Hardcoded problem shape (spec: nn_AttentionTypeEnsembleSheafLearner).
N, C, E, T, H, D = 50000, 128, 320000, 8, 64, 4
DD = D * D
EPS = 1e-5
P = 128
NCORES = 8
NLO = 32768          # dma_gather int16 index limit
GCHUNK = 768         # max idxs per dma_gather call (HW cap: 1024 crashes)
NSWQ = 4             # SWDGE queues — descriptor gen parallelizes across them
STRIP = 512          # edges per compute strip (PSUM bank = 512 f32)
SMCH = 16            # softmax mega-chunks

_PROGRAM_CACHE: dict = {}


def _plan_segments(tiles_tc):
    """tiles_tc: [T][4] tile counts. Returns (segments, ntp) where each
    segment is (t, cls, start_tile, n_tiles)."""
    segments = []
    pos = 0
    for t in range(T):
        for cls in range(4):
            n = tiles_tc[t][cls]
            if n:
                segments.append((t, cls, pos, n))
                pos += n
    if pos % 16:
        padt = 16 - pos % 16
        segments.append((7, 0, pos, padt))
        pos += padt
    return segments, pos


def _chunks(total, step):
    out = []
    o = 0
    while o < total:
        out.append((o, min(step, total - o)))
        o += step
    return out


def _build_program(segments, ntp, idxcols):
    import concourse.bacc as bacc
    import concourse.mybir as mybir
    import concourse.tile as tile
    from concourse.library_config import mlp

    f32 = mybir.dt.float32
    bf = mybir.dt.bfloat16
    i16 = mybir.dt.int16
    Alu = mybir.AluOpType
    Act = mybir.ActivationFunctionType
    X = mybir.AxisListType.X

    NE = ntp * P  # padded edge slots
    segmax = max(n for (_, _, _, n) in segments) * P

    nc = bacc.Bacc(
        None, target_bir_lowering=False, debug=False, num_swdge_queues=NSWQ,
        dynamic_dma_scratch_size=114688,
    )
    x_d = nc.declare_dram_parameter("xbf", [N, C], bf, isOutput=False)
    idx_d = nc.declare_dram_parameter("idxw", [P, idxcols], i16, isOutput=False)
    scal2_d = nc.declare_dram_parameter("scal2", [2, NE], bf, isOutput=False)
    std1_d = nc.declare_dram_parameter("std1", [1, NE], bf, isOutput=False)
    invc_d = nc.declare_dram_parameter("invc", [P, ntp], f32, isOutput=False)
    w1a_d = nc.declare_dram_parameter("w1a", [P, T * H], bf, isOutput=False)
    w1b_d = nc.declare_dram_parameter("w1b", [P, T * H], bf, isOutput=False)
    wc_d = nc.declare_dram_parameter("wc", [2, T * H], bf, isOutput=False)
    w2_d = nc.declare_dram_parameter("w2aug", [H + 1, T * DD], bf, isOutput=False)
    eye_d = nc.declare_dram_parameter("eyeb", [P, (ntp // SMCH) * DD], f32, isOutput=False)
    out_d = nc.declare_dram_parameter("out", [P, ntp * DD], f32, isOutput=True)

    x_lo = x_d[0:NLO, :]
    x_hi = x_d[NLO:N, :]

    gsems = [nc.alloc_semaphore(f"gsem{q}") for q in range(NSWQ)]
    with tile.TileContext(nc) as tc:
        with (
            tc.tile_pool(name="const", bufs=1) as cpool,
            tc.tile_pool(name="seg", bufs=4) as spool,
            tc.tile_pool(name="work", bufs=3) as wpool,
            tc.tile_pool(name="sm", bufs=2) as mpool,
            tc.tile_pool(name="psum", bufs=2, space="PSUM") as ppool,
            tc.tile_pool(name="psum2", bufs=4, space="PSUM") as ptpool,
        ):
            for q in range(NSWQ):
                nc.gpsimd.sem_clear(gsems[q])
            nc.gpsimd.load_library(mlp)
            idx_sb = cpool.tile([P, idxcols], i16)
            for c0 in range(0, idxcols, (idxcols + 7) // 8):
                c1 = min(idxcols, c0 + (idxcols + 7) // 8)
                nc.sync.dma_start(out=idx_sb[:, c0:c1], in_=idx_d[:, c0:c1])
            w1a_sb = cpool.tile([P, T * H], bf)
            nc.sync.dma_start(out=w1a_sb[:], in_=w1a_d[:, :])
            w1a_v = w1a_sb[:].rearrange("p (t h) -> p t h", t=T)
            w1b_sb = cpool.tile([P, T * H], bf)
            nc.sync.dma_start(out=w1b_sb[:], in_=w1b_d[:, :])
            w1b_v = w1b_sb[:].rearrange("p (t h) -> p t h", t=T)
            wc_sb = cpool.tile([2, T * H], bf)
            nc.sync.dma_start(out=wc_sb[:], in_=wc_d[:, :])
            wc_v = wc_sb[:].rearrange("p (t h) -> p t h", t=T)
            w2_sb = cpool.tile([H + 1, T * DD], bf)
            nc.sync.dma_start(out=w2_sb[:], in_=w2_d[:, :])
            w2_v = w2_sb[:].rearrange("p (t k) -> p t k", t=T)
            invc_sb = cpool.tile([P, ntp], f32)
            nc.sync.dma_start(out=invc_sb[:], in_=invc_d[:, :])
            eye_sb = cpool.tile([P, (ntp // SMCH) * DD], f32)
            nc.sync.dma_start(out=eye_sb[:], in_=eye_d[:, :])
            o2big = cpool.tile([P, ntp * DD], f32)
            o2v = o2big[:].rearrange("p (m k) -> p m k", m=ntp)

            icol = [0, 0]
            qload = [0] * NSWQ
            qcnt = [0] * NSWQ

            def gather(seg_tile, src_ap, ni, off):
                cw = ni // 16
                q = icol[1] % NSWQ
                inst = nc.gpsimd.dma_gather(
                    out_ap=seg_tile[:, off : off + ni].rearrange(
                        "p (a n) -> p a n", a=1
                    ),
                    in_ap=src_ap,
                    idxs_ap=idx_sb[:, icol[0] : icol[0] + cw],
                    num_idxs=ni,
                    num_idxs_reg=ni,
                    elem_size=C,
                    transpose=True,
                    queue_num=q,
                )
                inst.then_inc(gsems[q], 16)
                qcnt[q] += 1
                icol[0] += cw

            for si, (t, cls, tile0, ntile) in enumerate(segments):
                ne = ntile * P
                gbase = tile0 * P
                segr = spool.tile([P, segmax], bf, tag="segr")
                segc = spool.tile([P, segmax], bf, tag="segc")
                rsrc = x_lo if cls in (0, 1) else x_hi
                csrc = x_lo if cls in (0, 2) else x_hi
                icol[1] = min(range(NSWQ), key=lambda q: qload[q])
                qload[icol[1]] += ne
                qr = icol[1] % NSWQ
                for off, ni in _chunks(ne, GCHUNK):
                    gather(segr, rsrc, ni, off)
                cnt_r = qcnt[qr]
                icol[1] = min(range(NSWQ), key=lambda q: qload[q])
                qload[icol[1]] += ne
                qc = icol[1] % NSWQ
                for off, ni in _chunks(ne, GCHUNK):
                    gather(segc, csrc, ni, off)
                cnt_c = qcnt[qc]

                for soff, S in _chunks(ne, STRIP):
                    g0 = gbase + soff
                    if soff == 0:
                        # sound cross-queue RAW: per-queue completions are
                        # FIFO, so exact thresholds guarantee both gather
                        # groups have landed before PE consumes them
                        nc.tensor.wait_ge(gsems[qr], 16 * cnt_r)
                        if qc != qr:
                            nc.tensor.wait_ge(gsems[qc], 16 * cnt_c)
                    scalc = wpool.tile([2, STRIP], bf, tag="scalc")
                    nc.sync.dma_start(
                        out=scalc[:, 0:S], in_=scal2_d[:, g0 : g0 + S]
                    )
                    ps1 = ppool.tile([H, STRIP], f32, tag="ps1")
                    nc.tensor.matmul(
                        out=ps1[:, 0:S], lhsT=w1a_v[:, t, :],
                        rhs=segr[:, soff : soff + S], start=True, stop=False,
                    )
                    nc.tensor.matmul(
                        out=ps1[:, 0:S], lhsT=w1b_v[:, t, :],
                        rhs=segc[:, soff : soff + S], start=False, stop=False,
                    )
                    nc.tensor.matmul(
                        out=ps1[:, 0:S], lhsT=wc_v[:, t, :], rhs=scalc[:, 0:S],
                        start=False, stop=True,
                    )
                    h1 = wpool.tile([H + 1, STRIP], bf, tag="h1")
                    nc.scalar.activation(
                        out=h1[0:H, 0:S], in_=ps1[:, 0:S], func=Act.Relu
                    )
                    nc.sync.dma_start(
                        out=h1[H : H + 1, 0:S], in_=std1_d[:, g0 : g0 + S]
                    )
                    for i in range(S // P):
                        g = (g0 + i * P) // P
                        ps2 = ptpool.tile([P, DD], f32, tag="ps2")
                        nc.tensor.matmul(
                            out=ps2[:], lhsT=h1[:, i * P : (i + 1) * P],
                            rhs=w2_v[:, t, :], start=True, stop=True,
                        )
                        nc.vector.tensor_scalar(
                            out=o2v[:, g, :], in0=ps2[:],
                            scalar1=invc_sb[:, g : g + 1], scalar2=None,
                            op0=Alu.mult,
                        )

            # --- mega-batched softmax + (I - attn) ---
            mb = ntp // SMCH
            for s in range(SMCH):
                sl = o2v[:, s * mb : (s + 1) * mb, :]
                o4 = sl.rearrange("p m (i j) -> p m i j", i=D)
                mx = mpool.tile([P, mb, D], f32, tag="mx")
                nc.vector.tensor_reduce(out=mx[:], in_=o4, axis=X, op=Alu.max)
                sm = mpool.tile([P, mb * DD], f32, tag="sm")
                sm4 = sm[:].rearrange("p (m i j) -> p m i j", m=mb, i=D)
                nc.vector.tensor_tensor(
                    out=sm4, in0=o4,
                    in1=mx[:].unsqueeze(3).to_broadcast([P, mb, D, D]),
                    op=Alu.subtract,
                )
                nc.scalar.activation(out=sm[:], in_=sm[:], func=Act.Exp)
                sums = mpool.tile([P, mb, D], f32, tag="sums")
                nc.vector.tensor_reduce(out=sums[:], in_=sm4, axis=X, op=Alu.add)
                rec = mpool.tile([P, mb, D], f32, tag="rec")
                nc.vector.reciprocal(out=rec[:], in_=sums[:])
                nc.vector.tensor_tensor(
                    out=sm4, in0=sm4,
                    in1=rec[:].unsqueeze(3).to_broadcast([P, mb, D, D]),
                    op=Alu.mult,
                )
                outf = mpool.tile([P, mb * DD], f32, tag="outf")
                nc.vector.tensor_tensor(
                    out=outf[:], in0=eye_sb[:], in1=sm[:], op=Alu.subtract
                )
                nc.sync.dma_start(
                    out=out_d[:, s * mb * DD : (s + 1) * mb * DD], in_=outf[:]
                )
    nc.compile()
    return nc


def _wrap_idx(ids, ni):
    """dma_gather index layout: unwrapped[i] -> [i % 16, i // 16], replicated
    across the 8 Q7-core partition stripes -> [128, ni // 16] int16."""
    blk = ids.astype(np.uint16).reshape(ni // 16, 16).T
    return np.tile(blk, (8, 1)).astype(np.int16)


def _prepare(x, edge_index, edge_types, gamma, beta, W1, b1, W2, b2):
    x = np.asarray(x, dtype=np.float32)
    ei = np.asarray(edge_index).astype(np.int64)
    et = np.asarray(edge_types).astype(np.int64)
    gamma = np.asarray(gamma, dtype=np.float32)
    beta = np.asarray(beta, dtype=np.float32)
    W1 = np.asarray(W1, dtype=np.float32)
    b1 = np.asarray(b1, dtype=np.float32)
    W2 = np.asarray(W2, dtype=np.float32)
    b2 = np.asarray(b2, dtype=np.float32)

    # fold per-type affine (gamma/beta) into the first MLP layer
    W1e = gamma[:, :, None] * W1                      # [T, 2C, H]
    b1e = np.einsum("tc,tch->th", beta, W1) + b1      # [T, H]
    u = W1e.sum(axis=1)                               # [T, H]

    # per-edge LN stats from per-node partial sums (f64 for accuracy)
    s_node = x.sum(axis=1, dtype=np.float64)
    q_node = (x.astype(np.float64) ** 2).sum(axis=1)

    row, col = ei[0], ei[1]
    cls_all = (row >= NLO).astype(np.int64) * 2 + (col >= NLO).astype(np.int64)

    # per-core edge lists, grouped by (type, class), dealt round-robin
    percore = [[[None] * 4 for _ in range(T)] for _ in range(NCORES)]
    for t in range(T):
        for cl in range(4):
            sel = np.nonzero((et == t) & (cls_all == cl))[0]
            for k in range(NCORES):
                percore[k][t][cl] = sel[k::NCORES]
    tiles_tc = [
        [
            int(math.ceil(max(len(percore[k][t][cl]) for k in range(NCORES)) / P))
            for cl in range(4)
        ]
        for t in range(T)
    ]
    segments, ntp = _plan_segments(tiles_tc)
    NE = ntp * P

    # per-core edge slot assignment (first matching segment per (t, cl);
    # a trailing pad pseudo-segment may duplicate (7, 0) and stays empty)
    eids = np.full((NCORES, NE), -1, dtype=np.int64)
    for k in range(NCORES):
        for t in range(T):
            for cl in range(4):
                seg = next(
                    (s for s in segments if s[0] == t and s[1] == cl), None
                )
                if seg is None:
                    continue
                _, _, tile0, _ = seg
                arr = percore[k][t][cl]
                eids[k, tile0 * P : tile0 * P + len(arr)] = arr

    idxcols = sum(
        2 * sum(ni // 16 for (_, ni) in _chunks(n * P, GCHUNK))
        for (_, _, _, n) in segments
    )

    idx_host = np.zeros((NCORES, P, idxcols), dtype=np.int16)
    scal2_host = np.zeros((NCORES, 2, NE), dtype=bf16)
    std1_host = np.zeros((NCORES, 1, NE), dtype=bf16)
    invc_host = np.ones((NCORES, P, ntp), dtype=np.float32)

    for k in range(NCORES):
        e = eids[k]
        valid = e >= 0
        safe = np.where(valid, e, 0)
        r = np.where(valid, row[safe], 0)
        c = np.where(valid, col[safe], 0)
        ssum = s_node[r] + s_node[c]
        qsum = q_node[r] + q_node[c]
        mu = ssum / (2 * C)
        var = qsum / (2 * C) - mu * mu
        inv = 1.0 / np.sqrt(var + EPS)
        std = np.sqrt(var + EPS)
        mu = np.where(valid, mu, 0.0)
        inv = np.where(valid, inv, 1.0)
        std = np.where(valid, std, 1.0)

        scal2_host[k, 0, :] = (-mu).astype(bf16)
        scal2_host[k, 1, :] = std.astype(bf16)
        std1_host[k, 0, :] = std.astype(bf16)
        invc_host[k] = inv.astype(np.float32).reshape(ntp, P).T

        ic = 0
        for (t, cl, tile0, ntile) in segments:
            ne = ntile * P
            base = tile0 * P
            rr = r[base : base + ne].copy()
            cc = c[base : base + ne].copy()
            rr = rr - (NLO if cl in (2, 3) else 0)
            cc = cc - (NLO if cl in (1, 3) else 0)
            rr = np.maximum(rr, 0)
            cc = np.maximum(cc, 0)
            for off, ni in _chunks(ne, GCHUNK):
                idx_host[k, :, ic : ic + ni // 16] = _wrap_idx(rr[off : off + ni], ni)
                ic += ni // 16
            for off, ni in _chunks(ne, GCHUNK):
                idx_host[k, :, ic : ic + ni // 16] = _wrap_idx(cc[off : off + ni], ni)
                ic += ni // 16
        assert ic == idxcols

    x_bf = np.ascontiguousarray(x.astype(bf16))
    w1a_host = np.ascontiguousarray(
        W1e[:, :C, :].astype(bf16).transpose(1, 0, 2).reshape(P, T * H)
    )
    w1b_host = np.ascontiguousarray(
        W1e[:, C:, :].astype(bf16).transpose(1, 0, 2).reshape(P, T * H)
    )
    wc_host = np.ascontiguousarray(
        np.stack([u, b1e], axis=1).astype(bf16).transpose(1, 0, 2).reshape(2, T * H)
    )
    w2aug = np.concatenate([W2, b2[:, None, :]], axis=1)  # [T, H+1, DD]
    w2_host = np.ascontiguousarray(
        w2aug.astype(bf16).transpose(1, 0, 2).reshape(H + 1, T * DD)
    )
    mbt = ntp // SMCH
    eye_host = np.ascontiguousarray(
        np.broadcast_to(
            np.tile(np.eye(D, dtype=np.float32).reshape(DD), mbt), (P, mbt * DD)
        )
    )
    return dict(
        xbf=x_bf, idx=idx_host, scal2=scal2_host, std1=std1_host, invc=invc_host,
        w1a=w1a_host, w1b=w1b_host, wc=wc_host, w2aug=w2_host, eye=eye_host,
        eids=eids, segments=tuple(segments), ntp=ntp, idxcols=idxcols,
    )


_LAST_RESULTS = {}


def kernel(x, edge_index, edge_types, gamma, beta, W1, b1, W2, b2):
    from concourse.bass_utils import run_bass_kernel_spmd

    prep = _prepare(x, edge_index, edge_types, gamma, beta, W1, b1, W2, b2)
    segments, ntp, idxcols = prep["segments"], prep["ntp"], prep["idxcols"]

    key = (segments, ntp, idxcols)
    nc = _PROGRAM_CACHE.get(key)
    if nc is None:
        nc = _build_program(segments, ntp, idxcols)
        _PROGRAM_CACHE[key] = nc

    in_maps = [
        dict(
            xbf=prep["xbf"], idxw=prep["idx"][k], scal2=prep["scal2"][k],
            std1=prep["std1"][k], invc=prep["invc"][k], w1a=prep["w1a"],
            w1b=prep["w1b"], wc=prep["wc"], w2aug=prep["w2aug"], eyeb=prep["eye"],
        )
        for k in range(NCORES)
    ]
    trace = bool(int(os.environ.get("KERNEL_TRACE", "0")))
    res = run_bass_kernel_spmd(
        nc, in_maps, core_ids=list(range(NCORES)), trace=trace
    )
    _LAST_RESULTS["res"] = res

    out = np.zeros((E, DD), dtype=np.float32)
    for k in range(NCORES):
        o = (
            np.asarray(res.results[k]["out"])
            .reshape(P, ntp, DD)
            .transpose(1, 0, 2)
            .reshape(-1, DD)
        )
        e = prep["eids"][k]
        valid = e >= 0
        out[e[valid]] = o[valid]
    return out.reshape(E, D, D)
